# revision 53
# baseline (speedup 1.0000x reference)
# Trainium2 Bass kernel for nn_BatchelorAdj (motion-compensated MRI recon adjoint).
#
# Math:  out = sum_t W_t^T( sum_c conj(S_c) . IFFT2c(K_c . M_ct) )
#   - IFFT2c(X) == A @ X @ A with A = P F^-1 P (P = fftshift perm, A symmetric),
#     run as fp16 matmuls (1 cycle/row on PE) with fp32 PSUM accumulation.
#   - W_t^T (adjoint bilinear warp) == per 32x4 source tile a banded matmul
#     Ex^T @ (Ey*im): Ex[q,j] = relu(1-|j-px_q|), Ey[q,l] = relu(1-|l-py_q|).
#     Ex/Ey (the banded operator form of W_t, like A for the FFT) and the
#     masked k-space K.M are precomputed host-side; the device streams them
#     and does all matmuls, coil-combine and accumulation.
#
# Sharding: 8 cores; core r does frames [3r,3r+1,3r+2] fully, plus coils
# [2r,2r+1] of frame 24 (warp is linear in the image, so per-core partial coil
# sums warp independently and everything adds in the final host-side reduce).
import math
import numpy as np

Nx = Ny = 320
Nc = 16
Nt = 25
NCORES = 8
BX, BY = 32, 4              # warp source tile (BX*BY = 128 = one K chunk)
NTX, NTY = Nx // BX, Ny // BY
NTILE = NTX * NTY           # 800
FR_FULL = 3                 # full frames per core
C24 = Nc // NCORES          # coils of frame 24 per core
NSLOT = FR_FULL + 1
NPAIR = FR_FULL * Nc + C24  # (coil,frame) pairs per core (50)
QT = NTILE // 4             # warp chunk (200 tiles)

_CACHE = {}


def _build_A():
    j = np.arange(Nx)
    F = np.exp(2j * np.pi * np.outer(j, j) / Nx) / np.sqrt(Nx)
    P = np.zeros((Nx, Nx))
    P[j, (j + Nx // 2) % Nx] = 1.0
    A = P @ F @ P
    return A.real.astype(np.float32), A.imag.astype(np.float32)


def _chunk3(arr2d):
    """[320, W] -> [3, 128, W] zero-padded."""
    out = np.zeros((3, 128, arr2d.shape[1]), dtype=arr2d.dtype)
    out[0] = arr2d[0:128]
    out[1] = arr2d[128:256]
    out[2, :64] = arr2d[256:320]
    return out


def _build_program(D):
    from concourse import bass, bacc, tile, mybir

    JX = BX + 2 * D + 1
    JY = BY + 2 * D + 1
    PWW = BY * (NTY - 1) + JY          # psum band width (357 for D=18)
    f32 = mybir.dt.float32
    f16 = mybir.dt.float16
    MULT = mybir.AluOpType.mult
    ADD = mybir.AluOpType.add
    SUB = mybir.AluOpType.subtract

    nc = bacc.Bacc("TRN2", target_bir_lowering=False, debug=False,
                   num_devices=NCORES)

    # ---- DRAM tensors (SPMD: same shapes on all cores, per-core values) ----
    kmd = nc.dram_tensor("kmd", [NPAIR, 2, 3, 128, Nx], f16, kind="ExternalInput")
    smg = nc.dram_tensor("smg", [Nc, 2, 3, 128, Ny], f16, kind="ExternalInput")
    sm24g = nc.dram_tensor("sm24g", [C24, 2, 3, 128, Ny], f16, kind="ExternalInput")
    Acst = nc.dram_tensor("Acst", [3, 3, 128, Ny], f16, kind="ExternalInput")
    APcst = nc.dram_tensor("APcst", [2, 128, Ny], f16, kind="ExternalInput")
    exd = nc.dram_tensor("exd", [NSLOT, 4, 128, JX, QT], f16, kind="ExternalInput")
    eyd = nc.dram_tensor("eyd", [NSLOT, 4, 128, JY, QT], f16, kind="ExternalInput")
    zzd = nc.dram_tensor("zzd", [1, 512], f16, kind="ExternalInput")
    outp = nc.dram_tensor("outp", [2, 3, 128, Ny], f32, kind="ExternalOutput")

    from contextlib import ExitStack
    with tile.TileContext(nc) as tc, ExitStack() as ctx:
        const_pool = ctx.enter_context(tc.tile_pool(name="const", bufs=1))
        acc_pool = ctx.enter_context(tc.tile_pool(name="acc", bufs=1))
        aux_pool = ctx.enter_context(tc.tile_pool(name="aux", bufs=2))
        km_pool = ctx.enter_context(tc.tile_pool(name="km", bufs=2))
        t1_pool = ctx.enter_context(tc.tile_pool(name="t1", bufs=2))
        sm_pool = ctx.enter_context(tc.tile_pool(name="sm", bufs=2))
        pr_pool = ctx.enter_context(tc.tile_pool(name="pr", bufs=2))
        imc_pool = ctx.enter_context(tc.tile_pool(name="imc", bufs=2))
        axd_pool = ctx.enter_context(tc.tile_pool(name="axd", bufs=1))
        dx_pool = ctx.enter_context(tc.tile_pool(name="dx", bufs=2))
        ey_pool = ctx.enter_context(tc.tile_pool(name="ey", bufs=2))
        ei_pool = ctx.enter_context(tc.tile_pool(name="ei", bufs=2))
        fl_pool = ctx.enter_context(tc.tile_pool(name="fl", bufs=2))
        psum_fft = ctx.enter_context(tc.tile_pool(name="psf", bufs=4, space="PSUM"))
        psum_warp = ctx.enter_context(tc.tile_pool(name="psw", bufs=2, space="PSUM"))

        # ---- constants to SBUF ----
        A_sb = []
        for w in range(3):
            t = const_pool.tile([128, 3, Ny], f16, tag=f"A{w}")
            nc.sync.dma_start(t[:, :, :], Acst.ap()[w].transpose([1, 0, 2]))
            A_sb.append(t)
        AP_sb = []
        for w in range(2):
            t = const_pool.tile([128, Ny], f16, tag=f"AP{w}")
            nc.sync.dma_start(t[:, :], APcst.ap()[w])
            AP_sb.append(t)
        zzh_sb = const_pool.tile([1, 512], f16, tag="zzh")
        nc.sync.dma_start(zzh_sb[:, :], zzd.ap()[:, :])

        out_acc = []
        for comp in range(2):
            t = acc_pool.tile([128, 3, Ny], f32, tag=f"oacc{comp}")
            nc.vector.memset(t[:, :, :], 0.0)
            out_acc.append(t)

        # persistent pass-2 output tiles (fully memset once so full-tile
        # fp16 2x combine ops can read the m=2 padding rows)
        pv = []
        for comp in range(2):
            t = acc_pool.tile([128, 3, Ny], f16, tag=f"pv{comp}")
            nc.vector.memset(t[:, :, :], 0.0)
            pv.append(t)

        AR, AI, NAI = 0, 1, 2

        def fft_combine(pair, smbt, auxp):
            kmTt = []
            for comp in range(2):
                km = km_pool.tile([128, 3, Nx], f16, tag=f"km{comp}",
                                  name=f"km_{comp}")
                nc.sync.dma_start(km[:, :, :],
                                  kmd.ap()[pair, comp].transpose([1, 0, 2]))
                kmTt.append(km)

            # pass 1: T1 = km @ A  (T1[x,u], stored [p, m, u]) -> sbuf f16
            # km chunk2 ships packed [km_r tail; km_i tail], so the two
            # 64-row tail terms merge into one 128-row matmul against the
            # packed-A tiles AP_sb (5 matmuls per psum group instead of 6).
            # T1 chunk2 is packed the same way for pass 2.
            T1 = [t1_pool.tile([128, 3, Ny], f16, tag=f"T1{comp}",
                               name=f"T1_{comp}")
                  for comp in range(2)]
            for m in range(3):
                mc = 128 if m < 2 else 64
                for comp, terms in ((0, ((0, AR), (1, NAI))),
                                    (1, ((0, AI), (1, AR)))):
                    ps = psum_fft.tile([128, Ny], f32, tag="psf", name="psf")
                    i = 0
                    for (kcomp, w) in terms:
                        for ky in range(2):
                            nc.tensor.matmul(
                                ps[0:mc, :],
                                kmTt[kcomp][:, ky, 128 * m:128 * m + mc],
                                A_sb[w][:, ky, :],
                                start=(i == 0), stop=False)
                            i += 1
                    nc.tensor.matmul(
                        ps[0:mc, :],
                        kmTt[0][:, 2, 128 * m:128 * m + mc],
                        AP_sb[comp][:, :],
                        start=False, stop=True)
                    if comp == 0:
                        nc.scalar.copy(T1[0][0:mc, m, :], ps[0:mc, :])
                    elif m < 2:
                        nc.scalar.copy(T1[1][0:mc, m, :], ps[0:mc, :])
                    else:
                        nc.scalar.copy(T1[0][64:128, 2, :], ps[0:mc, :])

            # pass 2: im = A @ T1 -> psum -> Act copy into persistent pv
            for m in range(3):
                mc = 128 if m < 2 else 64
                for comp, terms in ((0, ((0, AR), (1, NAI))),
                                    (1, ((1, AR), (0, AI)))):
                    ps = psum_fft.tile([128, Ny], f32, tag="psf", name="psf2")
                    i = 0
                    for (tcomp, w) in terms:
                        for kx in range(2):
                            nc.tensor.matmul(
                                ps[0:mc, :],
                                A_sb[w][:, kx, 128 * m:128 * m + mc],
                                T1[tcomp][:, kx, :],
                                start=(i == 0), stop=False)
                            i += 1
                    ap_pk = AP_sb[0] if comp == 0 else AP_sb[1]
                    nc.tensor.matmul(
                        ps[0:mc, :],
                        ap_pk[:, 128 * m:128 * m + mc],
                        T1[0][:, 2, :],
                        start=False, stop=True)
                    nc.scalar.copy(
                        pv[comp][0:mc, m, :].rearrange(
                            "p (ul g) -> p g ul", ul=BY, g=NTY),
                        ps[0:mc, :].rearrange(
                            "p (g ul) -> p g ul", g=NTY, ul=BY))

            # combine: aux_r += sr*ir + si*ii ; aux_i += sr*ii - si*ir
            # full-tile fp16 TT (DVE 2x); m=2 padding rows are zeros.
            for (ocomp, scomp, icomp, op) in ((0, 0, 0, ADD), (0, 1, 1, ADD),
                                              (1, 0, 1, ADD), (1, 1, 0, SUB)):
                pr = pr_pool.tile([128, 3, Ny], f16, tag="pr", name="pr")
                nc.vector.tensor_tensor(pr[:, :, :], pv[icomp][:, :, :],
                                        smbt[scomp][:, :, :], MULT)
                nc.vector.tensor_tensor(auxp[ocomp][:, :, :],
                                        auxp[ocomp][:, :, :], pr[:, :, :], op)

        def load_sm(sm_src):
            smbt = []
            for comp in range(2):
                smb = sm_pool.tile([128, 3, Ny], f16, tag=f"smb{comp}",
                                   name=f"smb_{comp}")
                nc.sync.dma_start(smb[:, :, :], sm_src[comp].transpose([1, 0, 2]))
                smbt.append(smb)
            return smbt

        def flush(pw, bx):
            # ---- flush band: rows [BX*bx - D, BX*bx + BX + D] ----
            g0 = BX * bx - D
            r0, r1 = max(0, g0), min(Nx, g0 + JX)
            for comp in range(2):
                tmp = fl_pool.tile([JX, Ny], f32, tag="fl1")
                nc.scalar.copy(tmp[0:JX, :], pw[comp][0:JX, D:D + Ny])
                ra = r0
                while ra < r1:
                    k = ra // 128
                    rb = min(r1, 128 * (k + 1))
                    pa, pb = ra - 128 * k, rb - 128 * k
                    tmp2 = fl_pool.tile([128, Ny], f32, tag="fl2")
                    nc.gpsimd.memset(tmp2[:, :], 0.0)
                    nc.sync.dma_start(tmp2[pa:pb, :], tmp[ra - g0:rb - g0, :])
                    nc.vector.tensor_tensor(
                        out_acc[comp][:, k, :],
                        out_acc[comp][:, k, :], tmp2[:, :], ADD)
                    ra = rb

        # ---- deferred-emission warp units -------------------------------
        # Warp work for a finished slot is chopped into small emission
        # units interleaved between the NEXT group's FFT coils, so each
        # in-order engine queue alternates FFT and warp work and the PE
        # reaches warp matmuls with inputs already resident.
        def gather_unit(aux_t, imc, a):
            # 2 a-blocks of the imc gather per call
            with nc.allow_non_contiguous_dma(reason="imc gather"):
                for aa in (a, a + 1):
                    k, p0 = (32 * aa) // 128, (32 * aa) % 128
                    for comp in range(2):
                        axv = aux_t[comp].rearrange(
                            "p m (ul g) -> p m ul g", ul=BY, g=NTY)
                        for ul in range(BY):
                            eng = nc.sync if ul < 2 else nc.scalar
                            eng.dma_start(
                                imc[comp][32 * ul:32 * ul + 32,
                                          NTY * aa:NTY * aa + NTY],
                                axv[p0:p0 + 32, k, ul, :])

        def chunk_unit(slot, q4, imc, state):
            q0 = QT * q4                   # first global tile of chunk
            qs = slice(q0, q0 + QT)
            dx = dx_pool.tile([128, JX, QT], f16, tag="dx")
            nc.sync.dma_start(dx[:, :, :], exd.ap()[slot, q4])
            eyb = ey_pool.tile([128, JY, QT], f16, tag="eyb", name="eyb")
            nc.sync.dma_start(eyb[:, :, :], eyd.ap()[slot, q4])
            ey0 = ei_pool.tile([128, JY, QT], f16, tag="ey0", name="eyim_0")
            nc.vector.tensor_tensor(
                ey0[:, :, :], eyb[:, :, :],
                imc[0][:, qs].unsqueeze(1).broadcast_to([128, JY, QT]), MULT)
            nc.vector.tensor_tensor(
                eyb[:, :, :], eyb[:, :, :],
                imc[1][:, qs].unsqueeze(1).broadcast_to([128, JY, QT]), MULT)
            eyim = (ey0, eyb)
            for cq in range(QT):
                c = q0 + cq                # global tile index
                bx, ti = c // NTY, c % NTY
                if ti == 0:
                    state["pw"] = []
                    for comp in range(2):
                        t = psum_warp.tile([JX, PWW], f32, tag=f"pw{comp}",
                                           name=f"pw_{comp}")
                        nc.tensor.matmul(t[:, :], zzh_sb[0:1, 0:JX],
                                         zzh_sb[0:1, 0:PWW],
                                         start=True, stop=False,
                                         skip_group_check=True)
                        state["pw"].append(t)
                y0 = BY * ti
                last = (ti == NTY - 1)
                for comp in range(2):
                    nc.tensor.matmul(
                        state["pw"][comp][:, y0:y0 + JY],
                        dx[:, :, cq],
                        eyim[comp][:, :, cq],
                        start=False, stop=last,
                        skip_group_check=True)
                if last:
                    flush(state["pw"], bx)

        def warp_units(slot, aux_t):
            imc = [imc_pool.tile([128, NTILE], f16, tag=f"imc{comp}",
                                 name=f"imc_{comp}")
                   for comp in range(2)]
            state = {}
            units = []
            for a in range(0, NTX, 2):
                units.append(lambda a=a: gather_unit(aux_t, imc, a))
            for q4 in range(4):
                units.append(lambda q4=q4: chunk_unit(slot, q4, imc, state))
            return units

        aux = {}

        def make_aux(slot):
            pair_t = []
            for comp in range(2):
                t = aux_pool.tile([128, 3, Ny], f16, tag=f"aux{slot % 2}{comp}",
                                  name=f"aux_{slot}_{comp}")
                nc.vector.memset(t[:, :, :], 0.0)
                pair_t.append(t)
            aux[slot] = pair_t

        def do_fft(pairs, units):
            # pairs: list of (sm_ap, pair_idx, slot); interleave pending
            # warp units proportionally between coils.
            ui = 0
            for i, (sm_ap, pair, slot) in enumerate(pairs):
                smbt = load_sm(sm_ap)
                fft_combine(pair, smbt, aux[slot])
                while ui < len(units) * (i + 1) // len(pairs):
                    units[ui]()
                    ui += 1
            while ui < len(units):
                units[ui]()
                ui += 1

        make_aux(0)
        make_aux(1)
        p01 = []
        for c in range(Nc):
            p01.append((smg.ap()[c], 0 * Nc + c, 0))
            p01.append((smg.ap()[c], 1 * Nc + c, 1))
        do_fft(p01, [])
        u01 = warp_units(0, aux[0]) + warp_units(1, aux[1])
        make_aux(2)
        make_aux(3)
        p23 = [(smg.ap()[c], 2 * Nc + c, 2) for c in range(Nc)]
        p23 += [(sm24g.ap()[c], FR_FULL * Nc + c, 3) for c in range(C24)]
        do_fft(p23, u01)
        for u in warp_units(2, aux[2]) + warp_units(3, aux[3]):
            u()

        for comp in range(2):
            nc.sync.dma_start(outp.ap()[comp].transpose([1, 0, 2]),
                              out_acc[comp][:, :, :])

    nc.compile()
    return nc


def _host_prep(kspace_r, kspace_i, mask, smaps_r, smaps_i, flow, D):
    f32 = np.float32
    f16 = np.float16
    JX = BX + 2 * D + 1
    JY = BY + 2 * D + 1

    Ar, Ai = _build_A()
    Acst = np.stack([_chunk3(Ar), _chunk3(Ai), _chunk3(-Ai)]).astype(f16)
    # packed tail chunks: [A_r tail; -A_i tail] and [A_i tail; A_r tail]
    APcst = np.stack([
        np.concatenate([Ar[256:320], -Ai[256:320]], axis=0),
        np.concatenate([Ai[256:320], Ar[256:320]], axis=0)]).astype(f16)

    # masked k-space, transposed [t, c, comp, ychunk, p, x] f16;
    # chunk 2 of comp 0 is packed [km_r tail; km_i tail]
    kT = np.stack([kspace_r.transpose(2, 1, 0), kspace_i.transpose(2, 1, 0)], 1)
    mT = mask.transpose(3, 2, 1, 0)  # [t, c, y, x]
    kmf = kT[None] * mT[:, :, None]  # [t, c, comp, y, x]
    kmT = np.zeros((Nt, Nc, 2, 3, 128, Nx), f16)
    kmT[:, :, :, 0] = kmf[:, :, :, 0:128]
    kmT[:, :, :, 1] = kmf[:, :, :, 128:256]
    kmT[:, :, 0, 2, :64] = kmf[:, :, 0, 256:320]
    kmT[:, :, 0, 2, 64:128] = kmf[:, :, 1, 256:320]

    # smaps natural [c, comp, vchunk, p, u], then u deinterleaved to
    # [ul, g] (u = 4g + ul) to match the combine/warp aux layout
    sT = np.stack([smaps_r.transpose(2, 0, 1), smaps_i.transpose(2, 0, 1)], 1)
    smg = np.zeros((Nc, 2, 3, 128, Ny), f16)
    smg[:, :, 0] = sT[:, :, 0:128]
    smg[:, :, 1] = sT[:, :, 128:256]
    smg[:, :, 2, :64] = sT[:, :, 256:320]
    smg = np.ascontiguousarray(
        smg.reshape(Nc, 2, 3, 128, NTY, BY).transpose(0, 1, 2, 3, 5, 4)
        .reshape(Nc, 2, 3, 128, Ny))

    # warp weights: banded operator form of W_t^T per 32x4 source tile,
    # pixel layout q = yin*32 + xin, tile = bx*NTY + by.
    X, Y = np.meshgrid(np.arange(Nx, dtype=f32), np.arange(Ny, dtype=f32),
                       indexing="ij")
    bxg = np.repeat(np.arange(NTX), NTY).reshape(1, NTILE)
    byg = np.tile(np.arange(NTY), NTX).reshape(1, NTILE)
    jx = np.arange(JX, dtype=f32)
    jy = np.arange(JY, dtype=f32)
    ex_all = np.zeros((Nt, 4, 128, JX, QT), f16)
    ey_all = np.zeros((Nt, 4, 128, JY, QT), f16)
    for t in range(Nt):
        px = np.clip(X + flow[:, :, 0, t], 0.0, Nx - 1.0)
        py = np.clip(Y + flow[:, :, 1, t], 0.0, Ny - 1.0)
        pxc = px.reshape(NTX, BX, NTY, BY).transpose(3, 1, 0, 2).reshape(128, NTILE)
        pyc = py.reshape(NTX, BX, NTY, BY).transpose(3, 1, 0, 2).reshape(128, NTILE)
        pxr = pxc - BX * bxg + D
        pyr = pyc - BY * byg + D
        assert pxr.min() >= 0 and pxr.max() <= JX - 1 + 1e-3
        assert pyr.min() >= 0 and pyr.max() <= JY - 1 + 1e-3
        ex = np.maximum(0.0, 1.0 - np.abs(jx[None, :, None] - pxr[:, None, :]))
        ey = np.maximum(0.0, 1.0 - np.abs(jy[None, :, None] - pyr[:, None, :]))
        ex_all[t] = ex.reshape(128, JX, 4, QT).transpose(2, 0, 1, 3)
        ey_all[t] = ey.reshape(128, JY, 4, QT).transpose(2, 0, 1, 3)

    zz = np.zeros((1, 512), f16)

    in_maps = []
    for r in range(NCORES):
        fr = [FR_FULL * r + s for s in range(FR_FULL)]
        cs = [C24 * r + j for j in range(C24)]
        sel = fr + [Nt - 1]
        kmd = np.concatenate([
            kmT[fr].reshape(FR_FULL * Nc, 2, 3, 128, Nx),
            kmT[Nt - 1, cs]], axis=0)
        in_maps.append({
            "kmd": np.ascontiguousarray(kmd),
            "smg": smg,
            "sm24g": np.ascontiguousarray(smg[cs]),
            "Acst": Acst,
            "APcst": APcst,
            "exd": np.ascontiguousarray(ex_all[sel]),
            "eyd": np.ascontiguousarray(ey_all[sel]),
            "zzd": zz,
        })
    return in_maps


def kernel(kspace_r, kspace_i, mask, smaps_r, smaps_i, flow):
    from concourse.bass_utils import run_bass_kernel_spmd

    D = max(17, int(math.ceil(np.abs(flow).max())))
    if D not in _CACHE:
        _CACHE[D] = _build_program(D)
    nc = _CACHE[D]

    in_maps = _host_prep(np.asarray(kspace_r, np.float32),
                         np.asarray(kspace_i, np.float32),
                         np.asarray(mask, np.float32),
                         np.asarray(smaps_r, np.float32),
                         np.asarray(smaps_i, np.float32),
                         np.asarray(flow, np.float32), D)

    res = run_bass_kernel_spmd(nc, in_maps, core_ids=list(range(NCORES)))

    acc = np.zeros((2, Nx, Ny), np.float64)
    for r in range(NCORES):
        o = res.results[r]["outp"].astype(np.float64)  # [2, 3, 128, 320]
        for comp in range(2):
            acc[comp, 0:128] += o[comp, 0]
            acc[comp, 128:256] += o[comp, 1]
            acc[comp, 256:320] += o[comp, 2, :64]
    return np.stack([acc[0], acc[1]], axis=-1).astype(np.float32)


# revision 54
# speedup vs baseline: 1.0152x; 1.0152x over previous
# Trainium2 Bass kernel for nn_BatchelorAdj (motion-compensated MRI recon adjoint).
#
# Math:  out = sum_t W_t^T( sum_c conj(S_c) . IFFT2c(K_c . M_ct) )
#   - IFFT2c(X) == A @ X @ A with A = P F^-1 P (P = fftshift perm, A symmetric),
#     run as fp16 matmuls (1 cycle/row on PE) with fp32 PSUM accumulation.
#   - W_t^T (adjoint bilinear warp) == per 32x4 source tile a banded matmul
#     Ex^T @ (Ey*im): Ex[q,j] = relu(1-|j-px_q|), Ey[q,l] = relu(1-|l-py_q|).
#     Ex/Ey (the banded operator form of W_t, like A for the FFT) and the
#     masked k-space K.M are precomputed host-side; the device streams them
#     and does all matmuls, coil-combine and accumulation.
#
# Sharding: 8 cores; core r does frames [3r,3r+1,3r+2] fully, plus coils
# [2r,2r+1] of frame 24 (warp is linear in the image, so per-core partial coil
# sums warp independently and everything adds in the final host-side reduce).
import math
import numpy as np

Nx = Ny = 320
Nc = 16
Nt = 25
NCORES = 8
BX, BY = 32, 4              # warp source tile (BX*BY = 128 = one K chunk)
NTX, NTY = Nx // BX, Ny // BY
NTILE = NTX * NTY           # 800
FR_FULL = 3                 # full frames per core
C24 = Nc // NCORES          # coils of frame 24 per core
NSLOT = FR_FULL + 1
NPAIR = FR_FULL * Nc + C24  # (coil,frame) pairs per core (50)
QT = NTILE // 8             # warp chunk (100 tiles)

_CACHE = {}


def _build_A():
    j = np.arange(Nx)
    F = np.exp(2j * np.pi * np.outer(j, j) / Nx) / np.sqrt(Nx)
    P = np.zeros((Nx, Nx))
    P[j, (j + Nx // 2) % Nx] = 1.0
    A = P @ F @ P
    return A.real.astype(np.float32), A.imag.astype(np.float32)


def _chunk3(arr2d):
    """[320, W] -> [3, 128, W] zero-padded."""
    out = np.zeros((3, 128, arr2d.shape[1]), dtype=arr2d.dtype)
    out[0] = arr2d[0:128]
    out[1] = arr2d[128:256]
    out[2, :64] = arr2d[256:320]
    return out


def _build_program(D):
    from concourse import bass, bacc, tile, mybir

    JX = BX + 2 * D + 1
    JY = BY + 2 * D + 1
    PWW = BY * (NTY - 1) + JY          # psum band width (357 for D=18)
    f32 = mybir.dt.float32
    f16 = mybir.dt.float16
    MULT = mybir.AluOpType.mult
    ADD = mybir.AluOpType.add
    SUB = mybir.AluOpType.subtract

    nc = bacc.Bacc("TRN2", target_bir_lowering=False, debug=False,
                   num_devices=NCORES)

    # ---- DRAM tensors (SPMD: same shapes on all cores, per-core values) ----
    kmd = nc.dram_tensor("kmd", [NPAIR, 2, 3, 128, Nx], f16, kind="ExternalInput")
    smg = nc.dram_tensor("smg", [Nc, 2, 3, 128, Ny], f16, kind="ExternalInput")
    sm24g = nc.dram_tensor("sm24g", [C24, 2, 3, 128, Ny], f16, kind="ExternalInput")
    Acst = nc.dram_tensor("Acst", [3, 3, 128, Ny], f16, kind="ExternalInput")
    APcst = nc.dram_tensor("APcst", [2, 128, Ny], f16, kind="ExternalInput")
    exd = nc.dram_tensor("exd", [NSLOT, 8, 128, JX, QT], f16, kind="ExternalInput")
    eyd = nc.dram_tensor("eyd", [NSLOT, 8, 128, JY, QT], f16, kind="ExternalInput")
    zzd = nc.dram_tensor("zzd", [1, 512], f16, kind="ExternalInput")
    outp = nc.dram_tensor("outp", [2, 3, 128, Ny], f32, kind="ExternalOutput")

    from contextlib import ExitStack
    with tile.TileContext(nc) as tc, ExitStack() as ctx:
        const_pool = ctx.enter_context(tc.tile_pool(name="const", bufs=1))
        acc_pool = ctx.enter_context(tc.tile_pool(name="acc", bufs=1))
        aux_pool = ctx.enter_context(tc.tile_pool(name="aux", bufs=2))
        km_pool = ctx.enter_context(tc.tile_pool(name="km", bufs=2))
        t1_pool = ctx.enter_context(tc.tile_pool(name="t1", bufs=2))
        sm_pool = ctx.enter_context(tc.tile_pool(name="sm", bufs=2))
        pr_pool = ctx.enter_context(tc.tile_pool(name="pr", bufs=2))
        imc_pool = ctx.enter_context(tc.tile_pool(name="imc", bufs=2))
        axd_pool = ctx.enter_context(tc.tile_pool(name="axd", bufs=1))
        dx_pool = ctx.enter_context(tc.tile_pool(name="dx", bufs=3))
        ey_pool = ctx.enter_context(tc.tile_pool(name="ey", bufs=3))
        ei_pool = ctx.enter_context(tc.tile_pool(name="ei", bufs=3))
        fl_pool = ctx.enter_context(tc.tile_pool(name="fl", bufs=2))
        psum_fft = ctx.enter_context(tc.tile_pool(name="psf", bufs=4, space="PSUM"))
        psum_warp = ctx.enter_context(tc.tile_pool(name="psw", bufs=2, space="PSUM"))

        # ---- constants to SBUF ----
        A_sb = []
        for w in range(3):
            t = const_pool.tile([128, 3, Ny], f16, tag=f"A{w}")
            nc.sync.dma_start(t[:, :, :], Acst.ap()[w].transpose([1, 0, 2]))
            A_sb.append(t)
        AP_sb = []
        for w in range(2):
            t = const_pool.tile([128, Ny], f16, tag=f"AP{w}")
            nc.sync.dma_start(t[:, :], APcst.ap()[w])
            AP_sb.append(t)
        zzh_sb = const_pool.tile([1, 512], f16, tag="zzh")
        nc.sync.dma_start(zzh_sb[:, :], zzd.ap()[:, :])

        out_acc = []
        for comp in range(2):
            t = acc_pool.tile([128, 3, Ny], f32, tag=f"oacc{comp}")
            nc.vector.memset(t[:, :, :], 0.0)
            out_acc.append(t)

        # persistent pass-2 output tiles (fully memset once so full-tile
        # fp16 2x combine ops can read the m=2 padding rows)
        pv = []
        for comp in range(2):
            t = acc_pool.tile([128, 3, Ny], f16, tag=f"pv{comp}")
            nc.vector.memset(t[:, :, :], 0.0)
            pv.append(t)

        AR, AI, NAI = 0, 1, 2

        def fft_combine(pair, smbt, auxp):
            kmTt = []
            for comp in range(2):
                km = km_pool.tile([128, 3, Nx], f16, tag=f"km{comp}",
                                  name=f"km_{comp}")
                nc.sync.dma_start(km[:, :, :],
                                  kmd.ap()[pair, comp].transpose([1, 0, 2]))
                kmTt.append(km)

            # pass 1: T1 = km @ A  (T1[x,u], stored [p, m, u]) -> sbuf f16
            # km chunk2 ships packed [km_r tail; km_i tail], so the two
            # 64-row tail terms merge into one 128-row matmul against the
            # packed-A tiles AP_sb (5 matmuls per psum group instead of 6).
            # T1 chunk2 is packed the same way for pass 2.
            T1 = [t1_pool.tile([128, 3, Ny], f16, tag=f"T1{comp}",
                               name=f"T1_{comp}")
                  for comp in range(2)]
            for m in range(3):
                mc = 128 if m < 2 else 64
                for comp, terms in ((0, ((0, AR), (1, NAI))),
                                    (1, ((0, AI), (1, AR)))):
                    ps = psum_fft.tile([128, Ny], f32, tag="psf", name="psf")
                    i = 0
                    for (kcomp, w) in terms:
                        for ky in range(2):
                            nc.tensor.matmul(
                                ps[0:mc, :],
                                kmTt[kcomp][:, ky, 128 * m:128 * m + mc],
                                A_sb[w][:, ky, :],
                                start=(i == 0), stop=False)
                            i += 1
                    nc.tensor.matmul(
                        ps[0:mc, :],
                        kmTt[0][:, 2, 128 * m:128 * m + mc],
                        AP_sb[comp][:, :],
                        start=False, stop=True)
                    if comp == 0:
                        nc.scalar.copy(T1[0][0:mc, m, :], ps[0:mc, :])
                    elif m < 2:
                        nc.scalar.copy(T1[1][0:mc, m, :], ps[0:mc, :])
                    else:
                        nc.scalar.copy(T1[0][64:128, 2, :], ps[0:mc, :])

            # pass 2: im = A @ T1 -> psum -> Act copy into persistent pv
            for m in range(3):
                mc = 128 if m < 2 else 64
                for comp, terms in ((0, ((0, AR), (1, NAI))),
                                    (1, ((1, AR), (0, AI)))):
                    ps = psum_fft.tile([128, Ny], f32, tag="psf", name="psf2")
                    i = 0
                    for (tcomp, w) in terms:
                        for kx in range(2):
                            nc.tensor.matmul(
                                ps[0:mc, :],
                                A_sb[w][:, kx, 128 * m:128 * m + mc],
                                T1[tcomp][:, kx, :],
                                start=(i == 0), stop=False)
                            i += 1
                    ap_pk = AP_sb[0] if comp == 0 else AP_sb[1]
                    nc.tensor.matmul(
                        ps[0:mc, :],
                        ap_pk[:, 128 * m:128 * m + mc],
                        T1[0][:, 2, :],
                        start=False, stop=True)
                    nc.scalar.copy(
                        pv[comp][0:mc, m, :].rearrange(
                            "p (ul g) -> p g ul", ul=BY, g=NTY),
                        ps[0:mc, :].rearrange(
                            "p (g ul) -> p g ul", g=NTY, ul=BY))

            # combine: aux_r += sr*ir + si*ii ; aux_i += sr*ii - si*ir
            # full-tile fp16 TT (DVE 2x); m=2 padding rows are zeros.
            for (ocomp, scomp, icomp, op) in ((0, 0, 0, ADD), (0, 1, 1, ADD),
                                              (1, 0, 1, ADD), (1, 1, 0, SUB)):
                pr = pr_pool.tile([128, 3, Ny], f16, tag="pr", name="pr")
                nc.vector.tensor_tensor(pr[:, :, :], pv[icomp][:, :, :],
                                        smbt[scomp][:, :, :], MULT)
                nc.vector.tensor_tensor(auxp[ocomp][:, :, :],
                                        auxp[ocomp][:, :, :], pr[:, :, :], op)

        def load_sm(sm_src):
            smbt = []
            for comp in range(2):
                smb = sm_pool.tile([128, 3, Ny], f16, tag=f"smb{comp}",
                                   name=f"smb_{comp}")
                nc.sync.dma_start(smb[:, :, :], sm_src[comp].transpose([1, 0, 2]))
                smbt.append(smb)
            return smbt

        def flush(pw, bx):
            # ---- flush band: rows [BX*bx - D, BX*bx + BX + D] ----
            g0 = BX * bx - D
            r0, r1 = max(0, g0), min(Nx, g0 + JX)
            for comp in range(2):
                tmp = fl_pool.tile([JX, Ny], f32, tag="fl1")
                nc.scalar.copy(tmp[0:JX, :], pw[comp][0:JX, D:D + Ny])
                ra = r0
                while ra < r1:
                    k = ra // 128
                    rb = min(r1, 128 * (k + 1))
                    pa, pb = ra - 128 * k, rb - 128 * k
                    tmp2 = fl_pool.tile([128, Ny], f32, tag="fl2")
                    nc.gpsimd.memset(tmp2[:, :], 0.0)
                    nc.sync.dma_start(tmp2[pa:pb, :], tmp[ra - g0:rb - g0, :])
                    nc.vector.tensor_tensor(
                        out_acc[comp][:, k, :],
                        out_acc[comp][:, k, :], tmp2[:, :], ADD)
                    ra = rb

        # ---- deferred-emission warp units -------------------------------
        # Warp work for a finished slot is chopped into small emission
        # units interleaved between the NEXT group's FFT coils, so each
        # in-order engine queue alternates FFT and warp work and the PE
        # reaches warp matmuls with inputs already resident.
        def gather_unit(aux_t, imc, a):
            # 2 a-blocks of the imc gather per call
            with nc.allow_non_contiguous_dma(reason="imc gather"):
                for aa in (a, a + 1):
                    k, p0 = (32 * aa) // 128, (32 * aa) % 128
                    for comp in range(2):
                        axv = aux_t[comp].rearrange(
                            "p m (ul g) -> p m ul g", ul=BY, g=NTY)
                        for ul in range(BY):
                            eng = nc.sync if ul < 2 else nc.scalar
                            eng.dma_start(
                                imc[comp][32 * ul:32 * ul + 32,
                                          NTY * aa:NTY * aa + NTY],
                                axv[p0:p0 + 32, k, ul, :])

        def chunk_unit(slot, q4, imc, state):
            q0 = QT * q4                   # first global tile of chunk
            qs = slice(q0, q0 + QT)
            dx = dx_pool.tile([128, JX, QT], f16, tag="dx")
            nc.sync.dma_start(dx[:, :, :], exd.ap()[slot, q4])
            eyb = ey_pool.tile([128, JY, QT], f16, tag="eyb", name="eyb")
            nc.sync.dma_start(eyb[:, :, :], eyd.ap()[slot, q4])
            ey0 = ei_pool.tile([128, JY, QT], f16, tag="ey0", name="eyim_0")
            nc.vector.tensor_tensor(
                ey0[:, :, :], eyb[:, :, :],
                imc[0][:, qs].unsqueeze(1).broadcast_to([128, JY, QT]), MULT)
            nc.vector.tensor_tensor(
                eyb[:, :, :], eyb[:, :, :],
                imc[1][:, qs].unsqueeze(1).broadcast_to([128, JY, QT]), MULT)
            eyim = (ey0, eyb)
            for cq in range(QT):
                c = q0 + cq                # global tile index
                bx, ti = c // NTY, c % NTY
                if ti == 0:
                    state["pw"] = []
                    for comp in range(2):
                        t = psum_warp.tile([JX, PWW], f32, tag=f"pw{comp}",
                                           name=f"pw_{comp}")
                        nc.tensor.matmul(t[:, :], zzh_sb[0:1, 0:JX],
                                         zzh_sb[0:1, 0:PWW],
                                         start=True, stop=False,
                                         skip_group_check=True)
                        state["pw"].append(t)
                y0 = BY * ti
                last = (ti == NTY - 1)
                for comp in range(2):
                    nc.tensor.matmul(
                        state["pw"][comp][:, y0:y0 + JY],
                        dx[:, :, cq],
                        eyim[comp][:, :, cq],
                        start=False, stop=last,
                        skip_group_check=True)
                if last:
                    flush(state["pw"], bx)

        def warp_units(slot, aux_t):
            imc = [imc_pool.tile([128, NTILE], f16, tag=f"imc{comp}",
                                 name=f"imc_{comp}")
                   for comp in range(2)]
            state = {}
            units = []
            for a in range(0, NTX, 2):
                units.append(lambda a=a: gather_unit(aux_t, imc, a))
            for q4 in range(8):
                units.append(lambda q4=q4: chunk_unit(slot, q4, imc, state))
            return units

        aux = {}

        def make_aux(slot):
            pair_t = []
            for comp in range(2):
                t = aux_pool.tile([128, 3, Ny], f16, tag=f"aux{slot % 2}{comp}",
                                  name=f"aux_{slot}_{comp}")
                nc.vector.memset(t[:, :, :], 0.0)
                pair_t.append(t)
            aux[slot] = pair_t

        def do_fft(pairs, units):
            # pairs: list of (sm_ap, pair_idx, slot); interleave pending
            # warp units proportionally between coils.
            ui = 0
            for i, (sm_ap, pair, slot) in enumerate(pairs):
                smbt = load_sm(sm_ap)
                fft_combine(pair, smbt, aux[slot])
                while ui < len(units) * (i + 1) // len(pairs):
                    units[ui]()
                    ui += 1
            while ui < len(units):
                units[ui]()
                ui += 1

        make_aux(0)
        make_aux(1)
        p01 = []
        for c in range(Nc):
            p01.append((smg.ap()[c], 0 * Nc + c, 0))
            p01.append((smg.ap()[c], 1 * Nc + c, 1))
        do_fft(p01, [])
        u01 = warp_units(0, aux[0]) + warp_units(1, aux[1])
        make_aux(2)
        make_aux(3)
        p23 = [(smg.ap()[c], 2 * Nc + c, 2) for c in range(Nc)]
        p23 += [(sm24g.ap()[c], FR_FULL * Nc + c, 3) for c in range(C24)]
        do_fft(p23, u01)
        for u in warp_units(2, aux[2]) + warp_units(3, aux[3]):
            u()

        for comp in range(2):
            nc.sync.dma_start(outp.ap()[comp].transpose([1, 0, 2]),
                              out_acc[comp][:, :, :])

    nc.compile()
    return nc


def _host_prep(kspace_r, kspace_i, mask, smaps_r, smaps_i, flow, D):
    f32 = np.float32
    f16 = np.float16
    JX = BX + 2 * D + 1
    JY = BY + 2 * D + 1

    Ar, Ai = _build_A()
    Acst = np.stack([_chunk3(Ar), _chunk3(Ai), _chunk3(-Ai)]).astype(f16)
    # packed tail chunks: [A_r tail; -A_i tail] and [A_i tail; A_r tail]
    APcst = np.stack([
        np.concatenate([Ar[256:320], -Ai[256:320]], axis=0),
        np.concatenate([Ai[256:320], Ar[256:320]], axis=0)]).astype(f16)

    # masked k-space, transposed [t, c, comp, ychunk, p, x] f16;
    # chunk 2 of comp 0 is packed [km_r tail; km_i tail]
    kT = np.stack([kspace_r.transpose(2, 1, 0), kspace_i.transpose(2, 1, 0)], 1)
    mT = mask.transpose(3, 2, 1, 0)  # [t, c, y, x]
    kmf = kT[None] * mT[:, :, None]  # [t, c, comp, y, x]
    kmT = np.zeros((Nt, Nc, 2, 3, 128, Nx), f16)
    kmT[:, :, :, 0] = kmf[:, :, :, 0:128]
    kmT[:, :, :, 1] = kmf[:, :, :, 128:256]
    kmT[:, :, 0, 2, :64] = kmf[:, :, 0, 256:320]
    kmT[:, :, 0, 2, 64:128] = kmf[:, :, 1, 256:320]

    # smaps natural [c, comp, vchunk, p, u], then u deinterleaved to
    # [ul, g] (u = 4g + ul) to match the combine/warp aux layout
    sT = np.stack([smaps_r.transpose(2, 0, 1), smaps_i.transpose(2, 0, 1)], 1)
    smg = np.zeros((Nc, 2, 3, 128, Ny), f16)
    smg[:, :, 0] = sT[:, :, 0:128]
    smg[:, :, 1] = sT[:, :, 128:256]
    smg[:, :, 2, :64] = sT[:, :, 256:320]
    smg = np.ascontiguousarray(
        smg.reshape(Nc, 2, 3, 128, NTY, BY).transpose(0, 1, 2, 3, 5, 4)
        .reshape(Nc, 2, 3, 128, Ny))

    # warp weights: banded operator form of W_t^T per 32x4 source tile,
    # pixel layout q = yin*32 + xin, tile = bx*NTY + by.
    X, Y = np.meshgrid(np.arange(Nx, dtype=f32), np.arange(Ny, dtype=f32),
                       indexing="ij")
    bxg = np.repeat(np.arange(NTX), NTY).reshape(1, NTILE)
    byg = np.tile(np.arange(NTY), NTX).reshape(1, NTILE)
    jx = np.arange(JX, dtype=f32)
    jy = np.arange(JY, dtype=f32)
    ex_all = np.zeros((Nt, 8, 128, JX, QT), f16)
    ey_all = np.zeros((Nt, 8, 128, JY, QT), f16)
    for t in range(Nt):
        px = np.clip(X + flow[:, :, 0, t], 0.0, Nx - 1.0)
        py = np.clip(Y + flow[:, :, 1, t], 0.0, Ny - 1.0)
        pxc = px.reshape(NTX, BX, NTY, BY).transpose(3, 1, 0, 2).reshape(128, NTILE)
        pyc = py.reshape(NTX, BX, NTY, BY).transpose(3, 1, 0, 2).reshape(128, NTILE)
        pxr = pxc - BX * bxg + D
        pyr = pyc - BY * byg + D
        assert pxr.min() >= 0 and pxr.max() <= JX - 1 + 1e-3
        assert pyr.min() >= 0 and pyr.max() <= JY - 1 + 1e-3
        ex = np.maximum(0.0, 1.0 - np.abs(jx[None, :, None] - pxr[:, None, :]))
        ey = np.maximum(0.0, 1.0 - np.abs(jy[None, :, None] - pyr[:, None, :]))
        ex_all[t] = ex.reshape(128, JX, 8, QT).transpose(2, 0, 1, 3)
        ey_all[t] = ey.reshape(128, JY, 8, QT).transpose(2, 0, 1, 3)

    zz = np.zeros((1, 512), f16)

    in_maps = []
    for r in range(NCORES):
        fr = [FR_FULL * r + s for s in range(FR_FULL)]
        cs = [C24 * r + j for j in range(C24)]
        sel = fr + [Nt - 1]
        kmd = np.concatenate([
            kmT[fr].reshape(FR_FULL * Nc, 2, 3, 128, Nx),
            kmT[Nt - 1, cs]], axis=0)
        in_maps.append({
            "kmd": np.ascontiguousarray(kmd),
            "smg": smg,
            "sm24g": np.ascontiguousarray(smg[cs]),
            "Acst": Acst,
            "APcst": APcst,
            "exd": np.ascontiguousarray(ex_all[sel]),
            "eyd": np.ascontiguousarray(ey_all[sel]),
            "zzd": zz,
        })
    return in_maps


def kernel(kspace_r, kspace_i, mask, smaps_r, smaps_i, flow):
    from concourse.bass_utils import run_bass_kernel_spmd

    D = max(17, int(math.ceil(np.abs(flow).max())))
    if D not in _CACHE:
        _CACHE[D] = _build_program(D)
    nc = _CACHE[D]

    in_maps = _host_prep(np.asarray(kspace_r, np.float32),
                         np.asarray(kspace_i, np.float32),
                         np.asarray(mask, np.float32),
                         np.asarray(smaps_r, np.float32),
                         np.asarray(smaps_i, np.float32),
                         np.asarray(flow, np.float32), D)

    res = run_bass_kernel_spmd(nc, in_maps, core_ids=list(range(NCORES)))

    acc = np.zeros((2, Nx, Ny), np.float64)
    for r in range(NCORES):
        o = res.results[r]["outp"].astype(np.float64)  # [2, 3, 128, 320]
        for comp in range(2):
            acc[comp, 0:128] += o[comp, 0]
            acc[comp, 128:256] += o[comp, 1]
            acc[comp, 256:320] += o[comp, 2, :64]
    return np.stack([acc[0], acc[1]], axis=-1).astype(np.float32)


# revision 55
# speedup vs baseline: 1.0236x; 1.0083x over previous
# Trainium2 Bass kernel for nn_BatchelorAdj (motion-compensated MRI recon adjoint).
#
# Math:  out = sum_t W_t^T( sum_c conj(S_c) . IFFT2c(K_c . M_ct) )
#   - IFFT2c(X) == A @ X @ A with A = P F^-1 P (P = fftshift perm, A symmetric),
#     run as fp16 matmuls (1 cycle/row on PE) with fp32 PSUM accumulation.
#   - W_t^T (adjoint bilinear warp) == per 32x4 source tile a banded matmul
#     Ex^T @ (Ey*im): Ex[q,j] = relu(1-|j-px_q|), Ey[q,l] = relu(1-|l-py_q|).
#     Ex/Ey (the banded operator form of W_t, like A for the FFT) and the
#     masked k-space K.M are precomputed host-side; the device streams them
#     and does all matmuls, coil-combine and accumulation.
#
# Sharding: 8 cores; core r does frames [3r,3r+1,3r+2] fully, plus coils
# [2r,2r+1] of frame 24 (warp is linear in the image, so per-core partial coil
# sums warp independently and everything adds in the final host-side reduce).
import math
import numpy as np

Nx = Ny = 320
Nc = 16
Nt = 25
NCORES = 8
BX, BY = 32, 4              # warp source tile (BX*BY = 128 = one K chunk)
NTX, NTY = Nx // BX, Ny // BY
NTILE = NTX * NTY           # 800
FR_FULL = 3                 # full frames per core
C24 = Nc // NCORES          # coils of frame 24 per core
NSLOT = FR_FULL + 1
NPAIR = FR_FULL * Nc + C24  # (coil,frame) pairs per core (50)
QT = NTILE // 8             # warp chunk (100 tiles)

_CACHE = {}


def _build_A():
    j = np.arange(Nx)
    F = np.exp(2j * np.pi * np.outer(j, j) / Nx) / np.sqrt(Nx)
    P = np.zeros((Nx, Nx))
    P[j, (j + Nx // 2) % Nx] = 1.0
    A = P @ F @ P
    return A.real.astype(np.float32), A.imag.astype(np.float32)


def _chunk3(arr2d):
    """[320, W] -> [3, 128, W] zero-padded."""
    out = np.zeros((3, 128, arr2d.shape[1]), dtype=arr2d.dtype)
    out[0] = arr2d[0:128]
    out[1] = arr2d[128:256]
    out[2, :64] = arr2d[256:320]
    return out


def _build_program(D):
    from concourse import bass, bacc, tile, mybir

    JX = BX + 2 * D + 1
    JY = BY + 2 * D + 1
    PWW = BY * (NTY - 1) + JY          # psum band width (357 for D=18)
    f32 = mybir.dt.float32
    f16 = mybir.dt.float16
    MULT = mybir.AluOpType.mult
    ADD = mybir.AluOpType.add
    SUB = mybir.AluOpType.subtract

    nc = bacc.Bacc("TRN2", target_bir_lowering=False, debug=False,
                   num_devices=NCORES)

    # ---- DRAM tensors (SPMD: same shapes on all cores, per-core values) ----
    kmd = nc.dram_tensor("kmd", [NPAIR, 2, 3, 128, Nx], f16, kind="ExternalInput")
    smg = nc.dram_tensor("smg", [Nc, 2, 3, 128, Ny], f16, kind="ExternalInput")
    sm24g = nc.dram_tensor("sm24g", [C24, 2, 3, 128, Ny], f16, kind="ExternalInput")
    Acst = nc.dram_tensor("Acst", [3, 3, 128, Ny], f16, kind="ExternalInput")
    APcst = nc.dram_tensor("APcst", [2, 128, Ny], f16, kind="ExternalInput")
    exd = nc.dram_tensor("exd", [NSLOT, 8, 128, JX, QT], f16, kind="ExternalInput")
    eyd = nc.dram_tensor("eyd", [NSLOT, 8, 128, JY, QT], f16, kind="ExternalInput")
    zzd = nc.dram_tensor("zzd", [1, 512], f16, kind="ExternalInput")
    outp = nc.dram_tensor("outp", [2, 3, 128, Ny], f32, kind="ExternalOutput")

    from contextlib import ExitStack
    with tile.TileContext(nc) as tc, ExitStack() as ctx:
        const_pool = ctx.enter_context(tc.tile_pool(name="const", bufs=1))
        acc_pool = ctx.enter_context(tc.tile_pool(name="acc", bufs=1))
        aux_pool = ctx.enter_context(tc.tile_pool(name="aux", bufs=2))
        km_pool = ctx.enter_context(tc.tile_pool(name="km", bufs=2))
        t1_pool = ctx.enter_context(tc.tile_pool(name="t1", bufs=2))
        sm_pool = ctx.enter_context(tc.tile_pool(name="sm", bufs=2))
        pr_pool = ctx.enter_context(tc.tile_pool(name="pr", bufs=2))
        imc_pool = ctx.enter_context(tc.tile_pool(name="imc", bufs=2))
        axd_pool = ctx.enter_context(tc.tile_pool(name="axd", bufs=1))
        dx_pool = ctx.enter_context(tc.tile_pool(name="dx", bufs=4))
        ey_pool = ctx.enter_context(tc.tile_pool(name="ey", bufs=4))
        ei_pool = ctx.enter_context(tc.tile_pool(name="ei", bufs=4))
        fl_pool = ctx.enter_context(tc.tile_pool(name="fl", bufs=2))
        psum_fft = ctx.enter_context(tc.tile_pool(name="psf", bufs=4, space="PSUM"))
        psum_warp = ctx.enter_context(tc.tile_pool(name="psw", bufs=2, space="PSUM"))

        # ---- constants to SBUF ----
        A_sb = []
        for w in range(3):
            t = const_pool.tile([128, 3, Ny], f16, tag=f"A{w}")
            nc.sync.dma_start(t[:, :, :], Acst.ap()[w].transpose([1, 0, 2]))
            A_sb.append(t)
        AP_sb = []
        for w in range(2):
            t = const_pool.tile([128, Ny], f16, tag=f"AP{w}")
            nc.sync.dma_start(t[:, :], APcst.ap()[w])
            AP_sb.append(t)
        zzh_sb = const_pool.tile([1, 512], f16, tag="zzh")
        nc.sync.dma_start(zzh_sb[:, :], zzd.ap()[:, :])

        out_acc = []
        for comp in range(2):
            t = acc_pool.tile([128, 3, Ny], f32, tag=f"oacc{comp}")
            nc.vector.memset(t[:, :, :], 0.0)
            out_acc.append(t)

        # persistent pass-2 output tiles (fully memset once so full-tile
        # fp16 2x combine ops can read the m=2 padding rows)
        pv = []
        for comp in range(2):
            t = acc_pool.tile([128, 3, Ny], f16, tag=f"pv{comp}")
            nc.vector.memset(t[:, :, :], 0.0)
            pv.append(t)

        AR, AI, NAI = 0, 1, 2

        def fft_combine(pair, smbt, auxp):
            kmTt = []
            for comp in range(2):
                km = km_pool.tile([128, 3, Nx], f16, tag=f"km{comp}",
                                  name=f"km_{comp}")
                nc.sync.dma_start(km[:, :, :],
                                  kmd.ap()[pair, comp].transpose([1, 0, 2]))
                kmTt.append(km)

            # pass 1: T1 = km @ A  (T1[x,u], stored [p, m, u]) -> sbuf f16
            # km chunk2 ships packed [km_r tail; km_i tail], so the two
            # 64-row tail terms merge into one 128-row matmul against the
            # packed-A tiles AP_sb (5 matmuls per psum group instead of 6).
            # T1 chunk2 is packed the same way for pass 2.
            T1 = [t1_pool.tile([128, 3, Ny], f16, tag=f"T1{comp}",
                               name=f"T1_{comp}")
                  for comp in range(2)]
            for m in range(3):
                mc = 128 if m < 2 else 64
                for comp, terms in ((0, ((0, AR), (1, NAI))),
                                    (1, ((0, AI), (1, AR)))):
                    ps = psum_fft.tile([128, Ny], f32, tag="psf", name="psf")
                    i = 0
                    for (kcomp, w) in terms:
                        for ky in range(2):
                            nc.tensor.matmul(
                                ps[0:mc, :],
                                kmTt[kcomp][:, ky, 128 * m:128 * m + mc],
                                A_sb[w][:, ky, :],
                                start=(i == 0), stop=False)
                            i += 1
                    nc.tensor.matmul(
                        ps[0:mc, :],
                        kmTt[0][:, 2, 128 * m:128 * m + mc],
                        AP_sb[comp][:, :],
                        start=False, stop=True)
                    if comp == 0:
                        nc.scalar.copy(T1[0][0:mc, m, :], ps[0:mc, :])
                    elif m < 2:
                        nc.scalar.copy(T1[1][0:mc, m, :], ps[0:mc, :])
                    else:
                        nc.scalar.copy(T1[0][64:128, 2, :], ps[0:mc, :])

            # pass 2: im = A @ T1 -> psum -> Act copy into persistent pv
            for m in range(3):
                mc = 128 if m < 2 else 64
                for comp, terms in ((0, ((0, AR), (1, NAI))),
                                    (1, ((1, AR), (0, AI)))):
                    ps = psum_fft.tile([128, Ny], f32, tag="psf", name="psf2")
                    i = 0
                    for (tcomp, w) in terms:
                        for kx in range(2):
                            nc.tensor.matmul(
                                ps[0:mc, :],
                                A_sb[w][:, kx, 128 * m:128 * m + mc],
                                T1[tcomp][:, kx, :],
                                start=(i == 0), stop=False)
                            i += 1
                    ap_pk = AP_sb[0] if comp == 0 else AP_sb[1]
                    nc.tensor.matmul(
                        ps[0:mc, :],
                        ap_pk[:, 128 * m:128 * m + mc],
                        T1[0][:, 2, :],
                        start=False, stop=True)
                    nc.scalar.copy(
                        pv[comp][0:mc, m, :].rearrange(
                            "p (ul g) -> p g ul", ul=BY, g=NTY),
                        ps[0:mc, :].rearrange(
                            "p (g ul) -> p g ul", g=NTY, ul=BY))

            # combine: aux_r += sr*ir + si*ii ; aux_i += sr*ii - si*ir
            # full-tile fp16 TT (DVE 2x); m=2 padding rows are zeros.
            for (ocomp, scomp, icomp, op) in ((0, 0, 0, ADD), (0, 1, 1, ADD),
                                              (1, 0, 1, ADD), (1, 1, 0, SUB)):
                pr = pr_pool.tile([128, 3, Ny], f16, tag="pr", name="pr")
                nc.vector.tensor_tensor(pr[:, :, :], pv[icomp][:, :, :],
                                        smbt[scomp][:, :, :], MULT)
                nc.vector.tensor_tensor(auxp[ocomp][:, :, :],
                                        auxp[ocomp][:, :, :], pr[:, :, :], op)

        def load_sm(sm_src):
            smbt = []
            for comp in range(2):
                smb = sm_pool.tile([128, 3, Ny], f16, tag=f"smb{comp}",
                                   name=f"smb_{comp}")
                nc.sync.dma_start(smb[:, :, :], sm_src[comp].transpose([1, 0, 2]))
                smbt.append(smb)
            return smbt

        def flush(pw, bx):
            # ---- flush band: rows [BX*bx - D, BX*bx + BX + D] ----
            g0 = BX * bx - D
            r0, r1 = max(0, g0), min(Nx, g0 + JX)
            for comp in range(2):
                tmp = fl_pool.tile([JX, Ny], f32, tag="fl1")
                nc.scalar.copy(tmp[0:JX, :], pw[comp][0:JX, D:D + Ny])
                ra = r0
                while ra < r1:
                    k = ra // 128
                    rb = min(r1, 128 * (k + 1))
                    pa, pb = ra - 128 * k, rb - 128 * k
                    tmp2 = fl_pool.tile([128, Ny], f32, tag="fl2")
                    nc.gpsimd.memset(tmp2[:, :], 0.0)
                    nc.sync.dma_start(tmp2[pa:pb, :], tmp[ra - g0:rb - g0, :])
                    nc.vector.tensor_tensor(
                        out_acc[comp][:, k, :],
                        out_acc[comp][:, k, :], tmp2[:, :], ADD)
                    ra = rb

        # ---- deferred-emission warp units -------------------------------
        # Warp work for a finished slot is chopped into small emission
        # units interleaved between the NEXT group's FFT coils, so each
        # in-order engine queue alternates FFT and warp work and the PE
        # reaches warp matmuls with inputs already resident.
        def gather_unit(aux_t, imc, a):
            # 2 a-blocks of the imc gather per call
            with nc.allow_non_contiguous_dma(reason="imc gather"):
                for aa in (a, a + 1):
                    k, p0 = (32 * aa) // 128, (32 * aa) % 128
                    for comp in range(2):
                        axv = aux_t[comp].rearrange(
                            "p m (ul g) -> p m ul g", ul=BY, g=NTY)
                        for ul in range(BY):
                            eng = nc.sync if ul < 2 else nc.scalar
                            eng.dma_start(
                                imc[comp][32 * ul:32 * ul + 32,
                                          NTY * aa:NTY * aa + NTY],
                                axv[p0:p0 + 32, k, ul, :])

        def chunk_unit(slot, q4, imc, state):
            q0 = QT * q4                   # first global tile of chunk
            qs = slice(q0, q0 + QT)
            dx = dx_pool.tile([128, JX, QT], f16, tag="dx")
            nc.sync.dma_start(dx[:, :, :], exd.ap()[slot, q4])
            eyb = ey_pool.tile([128, JY, QT], f16, tag="eyb", name="eyb")
            nc.sync.dma_start(eyb[:, :, :], eyd.ap()[slot, q4])
            ey0 = ei_pool.tile([128, JY, QT], f16, tag="ey0", name="eyim_0")
            nc.vector.tensor_tensor(
                ey0[:, :, :], eyb[:, :, :],
                imc[0][:, qs].unsqueeze(1).broadcast_to([128, JY, QT]), MULT)
            nc.vector.tensor_tensor(
                eyb[:, :, :], eyb[:, :, :],
                imc[1][:, qs].unsqueeze(1).broadcast_to([128, JY, QT]), MULT)
            eyim = (ey0, eyb)
            for cq in range(QT):
                c = q0 + cq                # global tile index
                bx, ti = c // NTY, c % NTY
                if ti == 0:
                    state["pw"] = []
                    for comp in range(2):
                        t = psum_warp.tile([JX, PWW], f32, tag=f"pw{comp}",
                                           name=f"pw_{comp}")
                        nc.tensor.matmul(t[:, :], zzh_sb[0:1, 0:JX],
                                         zzh_sb[0:1, 0:PWW],
                                         start=True, stop=False,
                                         skip_group_check=True)
                        state["pw"].append(t)
                y0 = BY * ti
                last = (ti == NTY - 1)
                for comp in range(2):
                    nc.tensor.matmul(
                        state["pw"][comp][:, y0:y0 + JY],
                        dx[:, :, cq],
                        eyim[comp][:, :, cq],
                        start=False, stop=last,
                        skip_group_check=True)
                if last:
                    flush(state["pw"], bx)

        def warp_units(slot, aux_t):
            imc = [imc_pool.tile([128, NTILE], f16, tag=f"imc{comp}",
                                 name=f"imc_{comp}")
                   for comp in range(2)]
            state = {}
            units = []
            for a in range(0, NTX, 2):
                units.append(lambda a=a: gather_unit(aux_t, imc, a))
            for q4 in range(8):
                units.append(lambda q4=q4: chunk_unit(slot, q4, imc, state))
            return units

        aux = {}

        def make_aux(slot):
            pair_t = []
            for comp in range(2):
                t = aux_pool.tile([128, 3, Ny], f16, tag=f"aux{slot % 2}{comp}",
                                  name=f"aux_{slot}_{comp}")
                nc.vector.memset(t[:, :, :], 0.0)
                pair_t.append(t)
            aux[slot] = pair_t

        def do_fft(pairs, units):
            # pairs: list of (sm_ap, pair_idx, slot); interleave pending
            # warp units proportionally between coils.
            ui = 0
            for i, (sm_ap, pair, slot) in enumerate(pairs):
                smbt = load_sm(sm_ap)
                fft_combine(pair, smbt, aux[slot])
                while ui < len(units) * (i + 1) // len(pairs):
                    units[ui]()
                    ui += 1
            while ui < len(units):
                units[ui]()
                ui += 1

        make_aux(0)
        make_aux(1)
        p01 = []
        for c in range(Nc):
            p01.append((smg.ap()[c], 0 * Nc + c, 0))
            p01.append((smg.ap()[c], 1 * Nc + c, 1))
        do_fft(p01, [])
        u01 = warp_units(0, aux[0]) + warp_units(1, aux[1])
        make_aux(2)
        make_aux(3)
        p23 = [(smg.ap()[c], 2 * Nc + c, 2) for c in range(Nc)]
        p23 += [(sm24g.ap()[c], FR_FULL * Nc + c, 3) for c in range(C24)]
        do_fft(p23, u01)
        for u in warp_units(2, aux[2]) + warp_units(3, aux[3]):
            u()

        for comp in range(2):
            nc.sync.dma_start(outp.ap()[comp].transpose([1, 0, 2]),
                              out_acc[comp][:, :, :])

    nc.compile()
    return nc


def _host_prep(kspace_r, kspace_i, mask, smaps_r, smaps_i, flow, D):
    f32 = np.float32
    f16 = np.float16
    JX = BX + 2 * D + 1
    JY = BY + 2 * D + 1

    Ar, Ai = _build_A()
    Acst = np.stack([_chunk3(Ar), _chunk3(Ai), _chunk3(-Ai)]).astype(f16)
    # packed tail chunks: [A_r tail; -A_i tail] and [A_i tail; A_r tail]
    APcst = np.stack([
        np.concatenate([Ar[256:320], -Ai[256:320]], axis=0),
        np.concatenate([Ai[256:320], Ar[256:320]], axis=0)]).astype(f16)

    # masked k-space, transposed [t, c, comp, ychunk, p, x] f16;
    # chunk 2 of comp 0 is packed [km_r tail; km_i tail]
    kT = np.stack([kspace_r.transpose(2, 1, 0), kspace_i.transpose(2, 1, 0)], 1)
    mT = mask.transpose(3, 2, 1, 0)  # [t, c, y, x]
    kmf = kT[None] * mT[:, :, None]  # [t, c, comp, y, x]
    kmT = np.zeros((Nt, Nc, 2, 3, 128, Nx), f16)
    kmT[:, :, :, 0] = kmf[:, :, :, 0:128]
    kmT[:, :, :, 1] = kmf[:, :, :, 128:256]
    kmT[:, :, 0, 2, :64] = kmf[:, :, 0, 256:320]
    kmT[:, :, 0, 2, 64:128] = kmf[:, :, 1, 256:320]

    # smaps natural [c, comp, vchunk, p, u], then u deinterleaved to
    # [ul, g] (u = 4g + ul) to match the combine/warp aux layout
    sT = np.stack([smaps_r.transpose(2, 0, 1), smaps_i.transpose(2, 0, 1)], 1)
    smg = np.zeros((Nc, 2, 3, 128, Ny), f16)
    smg[:, :, 0] = sT[:, :, 0:128]
    smg[:, :, 1] = sT[:, :, 128:256]
    smg[:, :, 2, :64] = sT[:, :, 256:320]
    smg = np.ascontiguousarray(
        smg.reshape(Nc, 2, 3, 128, NTY, BY).transpose(0, 1, 2, 3, 5, 4)
        .reshape(Nc, 2, 3, 128, Ny))

    # warp weights: banded operator form of W_t^T per 32x4 source tile,
    # pixel layout q = yin*32 + xin, tile = bx*NTY + by.
    X, Y = np.meshgrid(np.arange(Nx, dtype=f32), np.arange(Ny, dtype=f32),
                       indexing="ij")
    bxg = np.repeat(np.arange(NTX), NTY).reshape(1, NTILE)
    byg = np.tile(np.arange(NTY), NTX).reshape(1, NTILE)
    jx = np.arange(JX, dtype=f32)
    jy = np.arange(JY, dtype=f32)
    ex_all = np.zeros((Nt, 8, 128, JX, QT), f16)
    ey_all = np.zeros((Nt, 8, 128, JY, QT), f16)
    for t in range(Nt):
        px = np.clip(X + flow[:, :, 0, t], 0.0, Nx - 1.0)
        py = np.clip(Y + flow[:, :, 1, t], 0.0, Ny - 1.0)
        pxc = px.reshape(NTX, BX, NTY, BY).transpose(3, 1, 0, 2).reshape(128, NTILE)
        pyc = py.reshape(NTX, BX, NTY, BY).transpose(3, 1, 0, 2).reshape(128, NTILE)
        pxr = pxc - BX * bxg + D
        pyr = pyc - BY * byg + D
        assert pxr.min() >= 0 and pxr.max() <= JX - 1 + 1e-3
        assert pyr.min() >= 0 and pyr.max() <= JY - 1 + 1e-3
        ex = np.maximum(0.0, 1.0 - np.abs(jx[None, :, None] - pxr[:, None, :]))
        ey = np.maximum(0.0, 1.0 - np.abs(jy[None, :, None] - pyr[:, None, :]))
        ex_all[t] = ex.reshape(128, JX, 8, QT).transpose(2, 0, 1, 3)
        ey_all[t] = ey.reshape(128, JY, 8, QT).transpose(2, 0, 1, 3)

    zz = np.zeros((1, 512), f16)

    in_maps = []
    for r in range(NCORES):
        fr = [FR_FULL * r + s for s in range(FR_FULL)]
        cs = [C24 * r + j for j in range(C24)]
        sel = fr + [Nt - 1]
        kmd = np.concatenate([
            kmT[fr].reshape(FR_FULL * Nc, 2, 3, 128, Nx),
            kmT[Nt - 1, cs]], axis=0)
        in_maps.append({
            "kmd": np.ascontiguousarray(kmd),
            "smg": smg,
            "sm24g": np.ascontiguousarray(smg[cs]),
            "Acst": Acst,
            "APcst": APcst,
            "exd": np.ascontiguousarray(ex_all[sel]),
            "eyd": np.ascontiguousarray(ey_all[sel]),
            "zzd": zz,
        })
    return in_maps


def kernel(kspace_r, kspace_i, mask, smaps_r, smaps_i, flow):
    from concourse.bass_utils import run_bass_kernel_spmd

    D = max(17, int(math.ceil(np.abs(flow).max())))
    if D not in _CACHE:
        _CACHE[D] = _build_program(D)
    nc = _CACHE[D]

    in_maps = _host_prep(np.asarray(kspace_r, np.float32),
                         np.asarray(kspace_i, np.float32),
                         np.asarray(mask, np.float32),
                         np.asarray(smaps_r, np.float32),
                         np.asarray(smaps_i, np.float32),
                         np.asarray(flow, np.float32), D)

    res = run_bass_kernel_spmd(nc, in_maps, core_ids=list(range(NCORES)))

    acc = np.zeros((2, Nx, Ny), np.float64)
    for r in range(NCORES):
        o = res.results[r]["outp"].astype(np.float64)  # [2, 3, 128, 320]
        for comp in range(2):
            acc[comp, 0:128] += o[comp, 0]
            acc[comp, 128:256] += o[comp, 1]
            acc[comp, 256:320] += o[comp, 2, :64]
    return np.stack([acc[0], acc[1]], axis=-1).astype(np.float32)
